# revision 12
# baseline (speedup 1.0000x reference)
"""Trainium2 Bass kernel for nn_Complex_CA (8-step cellular automaton).

Sharding: 8 cores x 128-row bands with 20-row ghost zones (no per-step
communication; validity shrinks 2 rows/step, margin absorbs it).

Per-core layout: 6 row-blocks of 32 (28 interior + 2+2 halo), columns padded
to 1026 with zero guards.  SBUF state tile xs[128, 6*1026]: partition =
32*ci + rr (ci: 0..2 = state channels, 3 = static scent), free = (block, col).

Per step: 3x3 conv via banded fp32r matmuls (4 ci row-tiled concurrent,
K=32 quadrant windows), 1x1 convs via (ch,row)-packed K=128 matmuls,
maxpool>0.1 via count-dilation matmuls on thresholded bits, life mask =
min(min(pre,1),post), state rebuilt to quadrant layout via permutation
matmuls, clip fused into the rebuild evacuation.
"""
import sys
import numpy as np

sys.path.insert(0, "/opt/trn_rl_repo")

H = W = 1024
NCORES = 8
NB = 6            # row blocks per core
BI = 28           # interior rows per block
BR = 32           # rows per block (with halo)
WG = 1026         # padded width (1 guard col each side)
TOP = 20          # ghost rows above the core band
FREE = NB * WG


def _build_tables(w2, w3, w4):
    """Host-built stationary matrices for all matmuls (fp32)."""
    t = {}
    # conv1: lhsT1[(h,dx,ci)] [32,128]: [rr_in, 16*co+ro] = w2[co,ci,dy,dx]
    L1 = np.zeros((2, 3, 4, 32, 128), np.float32)
    for h in range(2):
        for dx in range(3):
            for ci in range(4):
                for co in range(8):
                    for ro in range(16):
                        for dy in range(3):
                            ri = 16 * h + ro + dy - 1
                            if 0 <= ri < 32:
                                L1[h, dx, ci, ri, 16 * co + ro] = w2[co, ci, dy, dx]
    L1f = np.zeros((128, 6 * 128), np.float32)
    for h in range(2):
        for dx in range(3):
            blk = h * 3 + dx
            for ci in range(4):
                L1f[32 * ci:32 * ci + 32, 128 * blk:128 * blk + 128] = L1[h, dx, ci]
    t["L1"] = L1f

    # conv2: [128,128]: [16*ci+ri, 16*co+ro] = w3[co,ci]*d(ri==ro)
    L2 = np.zeros((128, 128), np.float32)
    for ci in range(8):
        for co in range(8):
            for r in range(16):
                L2[16 * ci + r, 16 * co + r] = w3[co, ci, 0, 0]
    t["L2"] = L2

    # conv3: [128,48]: [16*ci+ri, 16*s+ro] = w4[s,ci]*d(ri==ro), s<3
    L3 = np.zeros((128, 48), np.float32)
    for ci in range(8):
        for s in range(3):
            for r in range(16):
                L3[16 * ci + r, 16 * s + r] = w4[s, ci, 0, 0]
    t["L3"] = L3

    # residual passthrough: Lres[(h,s)] [32,48]: [rr, 16*s+ro] = d(rr==16h+ro)
    Lres = np.zeros((2, 3, 32, 48), np.float32)
    for h in range(2):
        for s in range(3):
            for ro in range(16):
                Lres[h, s, 16 * h + ro, 16 * s + ro] = 1.0
    Lresf = np.zeros((128, 96), np.float32)
    for h in range(2):
        for s in range(3):
            Lresf[32 * s:32 * s + 32, 48 * h:48 * h + 48] = Lres[h, s]
    t["Lres"] = Lresf

    # pre-mask dilate: [32,64]: [ri, 32h+ro] = 1 if |ri-(16h+ro)|<=1
    Lpre = np.zeros((32, 64), np.float32)
    for h in range(2):
        for ro in range(16):
            for dy in (-1, 0, 1):
                ri = 16 * h + ro + dy
                if 0 <= ri < 32:
                    Lpre[ri, 32 * h + ro] = 1.0
    t["Lpre"] = Lpre

    # post-mask dilate split-K: Lpost[hs] [16,64]: [ri, 32h+ro]=1 if |16hs+ri-(16h+ro)|<=1
    Lpost = np.zeros((2, 16, 64), np.float32)
    for hs in range(2):
        for h in range(2):
            for ro in range(16):
                for dy in (-1, 0, 1):
                    ri = 16 * h + ro + dy - 16 * hs
                    if 0 <= ri < 16:
                        Lpost[hs, ri, 32 * h + ro] = 1.0
    t["Lpost"] = Lpost.transpose(1, 0, 2).reshape(16, 128).copy()

    # life replicate: [16,48] = [I|I|I]
    Lrep = np.zeros((16, 48), np.float32)
    for s in range(3):
        for r in range(16):
            Lrep[r, 16 * s + r] = 1.0
    t["Lrep"] = Lrep

    # rebuild: Lrb[h] [48,128]: [16*s+ro, 32*ci+rr] = d(ci==s)*d(rr==16h+ro)
    Lrb = np.zeros((2, 48, 128), np.float32)
    for h in range(2):
        for s in range(3):
            for ro in range(16):
                Lrb[h, 16 * s + ro, 32 * s + 16 * h + ro] = 1.0
    t["Lrb"] = Lrb.transpose(1, 0, 2).reshape(48, 256).copy()
    return t


def _scent_host(food):
    f = np.asarray(food, np.float32)
    p = np.pad(f, 1)
    s = np.zeros_like(f)
    k = np.array([[0.25, 0.5, 0.25], [0.5, 1.0, 0.5], [0.25, 0.5, 0.25]], np.float32)
    for dy in range(3):
        for dx in range(3):
            s += k[dy, dx] * p[dy:dy + H, dx:dx + W]
    return s


def _build_xs(cell, scent, core):
    """Per-core initial xs [128, NB*WG] (quadrant layout, zero guards)."""
    chans = np.stack([cell[0], cell[1], cell[2], scent])  # [4,H,W]
    r0 = 128 * core - TOP
    g = r0 - 2 + BI * np.arange(NB)[:, None] + np.arange(BR)[None, :]  # [NB,BR]
    ok = (g >= 0) & (g < H)
    gc = np.clip(g, 0, H - 1)
    arr = chans[:, gc, :] * ok[None, :, :, None]          # [4,NB,BR,W]
    xs = np.zeros((4, BR, NB, WG), np.float32)
    xs[:, :, :, 1:1 + W] = arr.transpose(0, 2, 1, 3)
    return xs.reshape(128, NB * WG)


def _build_program(steps, biases_zero, stage=99, use_loop=False):
    import concourse.bass as bass
    import concourse.mybir as mybir
    import concourse.tile as tile

    dt = mybir.dt
    Alu = mybir.AluOpType
    Act = mybir.ActivationFunctionType
    f32 = dt.float32
    f32r = dt.float32r

    nc = bass.Bass("TRN2", target_bir_lowering=False)
    d_xs = nc.dram_tensor("xs0", [128, FREE], f32, kind="ExternalInput")
    d_L1 = nc.dram_tensor("L1", [128, 6 * 128], f32, kind="ExternalInput")
    d_L2 = nc.dram_tensor("L2", [128, 128], f32, kind="ExternalInput")
    d_L3 = nc.dram_tensor("L3", [128, 48], f32, kind="ExternalInput")
    d_Lres = nc.dram_tensor("Lres", [128, 96], f32, kind="ExternalInput")
    d_Lpre = nc.dram_tensor("Lpre", [32, 64], f32r, kind="ExternalInput")
    d_Lpost = nc.dram_tensor("Lpost", [16, 128], f32r, kind="ExternalInput")
    d_Lrep = nc.dram_tensor("Lrep", [16, 48], f32r, kind="ExternalInput")
    d_Lrb = nc.dram_tensor("Lrb", [48, 256], f32, kind="ExternalInput")
    d_bias = nc.dram_tensor("biasv", [128, 3], f32, kind="ExternalInput")
    d_vm = nc.dram_tensor("vm", [96, 2 * WG], f32, kind="ExternalInput")
    d_out = nc.dram_tensor("xs_out", [128, FREE], f32, kind="ExternalOutput")

    with tile.TileContext(nc) as tc:
        with tc.tile_pool(name="persist", bufs=1) as pp, \
             tc.tile_pool(name="work", bufs=2) as wp, \
             tc.tile_pool(name="psum", bufs=4, space="PSUM") as psp:
            xs = pp.tile([128, FREE], f32)
            nc.sync.dma_start(xs[:], d_xs[:])
            L1 = pp.tile([128, 6 * 128], f32)
            nc.sync.dma_start(L1[:], d_L1[:])
            L2 = pp.tile([128, 128], f32)
            nc.sync.dma_start(L2[:], d_L2[:])
            L3 = pp.tile([128, 48], f32)
            nc.sync.dma_start(L3[:], d_L3[:])
            Lres = pp.tile([128, 96], f32)
            nc.sync.dma_start(Lres[:], d_Lres[:])
            Lpre = pp.tile([32, 64], f32r)
            nc.sync.dma_start(Lpre[:], d_Lpre[:])
            Lpost = pp.tile([16, 128], f32r)
            nc.sync.dma_start(Lpost[:], d_Lpost[:])
            Lrep = pp.tile([16, 48], f32r)
            nc.sync.dma_start(Lrep[:], d_Lrep[:])
            Lrb = pp.tile([48, 256], f32)
            nc.sync.dma_start(Lrb[:], d_Lrb[:])
            bias = pp.tile([128, 3], f32)
            nc.sync.dma_start(bias[:], d_bias[:])
            vm = pp.tile([96, 2 * WG], f32)
            nc.sync.dma_start(vm[:], d_vm[:])

            b0 = pp.tile([32, FREE], f32r)       # pre-mask bits
            bp = pp.tile([16, NB * 2 * WG], f32r)  # post-mask bits (h-split rows)
            nc.vector.memset(bp[:].bitcast(f32), 0.0)

            def mm(out, lhsT, rhs, start, stop, tp=(0, 0)):
                nc.tensor.matmul(out, lhsT, rhs,
                                 start=start, stop=stop, tile_position=tp)

            import contextlib

            def step_iter():
                if use_loop:
                    return [tc.For_i(0, steps, 1)]
                return [contextlib.nullcontext() for _ in range(steps)]

            for _ctx in step_iter():
                if stage < 1:
                    break
                _ctx.__enter__()
                # ---- pre-mask bits from xs ch0 (guards stay 0 automatically)
                nc.gpsimd.tensor_scalar(b0[:], xs[0:32, :], 0.1, None, Alu.is_gt)

                for b in range(NB):
                    if stage < 2:
                        break
                    fb = b * WG
                    # ---- pre-mask count conv
                    ps_pre = psp.tile([128, 1024], f32, tag="ps")
                    for k in range(2):
                        for dx in range(3):
                            mm(ps_pre[0:64, 512 * k:512 * (k + 1)], Lpre[:],
                               b0[0:32, fb + dx + 512 * k: fb + dx + 512 * k + 512],
                               start=(dx == 0), stop=(dx == 2))
                    pre16 = wp.tile([16, 2048], f32, tag="pre16")
                    for h in range(2):
                        nc.scalar.activation(pre16[0:16, 1024 * h:1024 * (h + 1)],
                                             ps_pre[32 * h:32 * h + 16, :], Act.Copy)
                    if stage < 3:
                        continue

                    ps3s = []
                    for h in range(2):
                        # ---- conv1 (3x3, 4->8) banded, 4 ci row-tiled
                        ps1 = psp.tile([128, 1024], f32, tag="ps")
                        for k in range(2):
                            for dx in range(3):
                                col = (h * 3 + dx) * 128
                                mm(ps1[:, 512 * k:512 * (k + 1)],
                                   L1[:, col:col + 128],
                                   xs[:, fb + dx + 512 * k: fb + dx + 512 * k + 512],
                                   start=(dx == 0), stop=(dx == 2))
                        y8 = wp.tile([128, 1024], f32, tag="y8")
                        if biases_zero:
                            nc.scalar.activation(y8[:], ps1[:], Act.Relu)
                        else:
                            nc.scalar.activation(y8[:], ps1[:], Act.Relu,
                                                 bias=bias[0:128, 0:1])
                        if stage < 4:
                            ps3s.append(ps1)
                            continue
                        # ---- conv2 (1x1, 8->8)
                        ps2 = psp.tile([128, 1024], f32, tag="ps")
                        for k in range(2):
                            mm(ps2[:, 512 * k:512 * (k + 1)], L2[:],
                               y8[:, 512 * k:512 * (k + 1)], start=True, stop=True)
                        y2 = wp.tile([128, 1024], f32, tag="y2")
                        if biases_zero:
                            nc.scalar.activation(y2[:], ps2[:], Act.Relu)
                        else:
                            nc.scalar.activation(y2[:], ps2[:], Act.Relu,
                                                 bias=bias[0:128, 1:2])
                        if stage < 5:
                            ps3s.append(ps2)
                            continue
                        # ---- conv3 (1x1, 8->3) + residual passthrough
                        ps3 = psp.tile([128, 1024], f32, tag="ps")
                        for k in range(2):
                            mm(ps3[0:48, 512 * k:512 * (k + 1)], L3[:],
                               y2[:, 512 * k:512 * (k + 1)], start=True, stop=False)
                            mm(ps3[0:48, 512 * k:512 * (k + 1)],
                               Lres[:, 48 * h:48 * h + 48],
                               xs[:, fb + 1 + 512 * k: fb + 1 + 512 * k + 512],
                               start=False, stop=True)
                        # ---- post-mask bits from y3 ch0
                        nc.vector.tensor_scalar(
                            bp[0:16, (b * 2 + h) * WG + 1:(b * 2 + h) * WG + 1025],
                            ps3[0:16, :], 0.1, None, Alu.is_gt)
                        ps3s.append(ps3)
                    if stage < 6:
                        continue

                    # ---- post-mask count conv (split-K over h-halves)
                    ps_post = psp.tile([128, 1024], f32, tag="ps")
                    fb2 = b * 2 * WG
                    for k in range(2):
                        for dx in range(3):
                            for hs in range(2):
                                mm(ps_post[0:64, 512 * k:512 * (k + 1)],
                                   Lpost[:, 64 * hs:64 * hs + 64],
                                   bp[0:16, fb2 + hs * WG + dx + 512 * k:
                                      fb2 + hs * WG + dx + 512 * k + 512],
                                   start=(dx == 0 and hs == 0),
                                   stop=(dx == 2 and hs == 1))
                    # ---- life = min(min(pre,1), post)  in {0,1}
                    life16 = wp.tile([16, 2048], f32r, tag="life16")
                    for h in range(2):
                        nc.vector.scalar_tensor_tensor(
                            life16[0:16, 1024 * h:1024 * (h + 1)],
                            pre16[0:16, 1024 * h:1024 * (h + 1)], 1.0,
                            ps_post[32 * h:32 * h + 16, :], Alu.min, Alu.min)
                    if stage < 7:
                        continue
                    # ---- replicate life across 3 channels via PE
                    xn = wp.tile([48, 2048], f32, tag="xn")
                    for h in range(2):
                        ps_l = psp.tile([128, 1024], f32, tag="ps")
                        for k in range(2):
                            mm(ps_l[0:48, 512 * k:512 * (k + 1)], Lrep[:],
                               life16[0:16, 1024 * h + 512 * k:
                                      1024 * h + 512 * k + 512],
                               start=True, stop=True)
                        lf = wp.tile([48, 1024], f32, tag="lf")
                        nc.scalar.activation(lf[:], ps_l[0:48, :], Act.Copy)
                        # ---- apply: xn = (ps3 + b4) * life
                        if biases_zero:
                            nc.vector.scalar_tensor_tensor(
                                xn[0:48, 1024 * h:1024 * (h + 1)],
                                ps3s[h][0:48, :], 1.0, lf[:], Alu.mult, Alu.mult)
                        else:
                            nc.vector.scalar_tensor_tensor(
                                xn[0:48, 1024 * h:1024 * (h + 1)],
                                ps3s[h][0:48, :], bias[0:48, 2:3], lf[:],
                                Alu.add, Alu.mult)
                    if stage < 8:
                        continue
                    # ---- rebuild xs quadrant layout via permutation matmuls
                    ps_x = psp.tile([128, 1024], f32, tag="ps")
                    for k in range(2):
                        for h in range(2):
                            mm(ps_x[:, 512 * k:512 * (k + 1)],
                               Lrb[:, 128 * h:128 * (h + 1)],
                               xn[0:48, 1024 * h + 512 * k:1024 * h + 512 * k + 512],
                               start=(h == 0), stop=(h == 1))
                    # clip fused into the evacuation
                    nc.vector.tensor_scalar(xs[0:96, fb + 1:fb + 1025],
                                            ps_x[0:96, :], 10.0, -10.0,
                                            Alu.min, Alu.max)

                # ---- zero ghost rows at the global image boundary (edge blocks)
                nc.vector.tensor_tensor(xs[0:96, 0:WG], xs[0:96, 0:WG],
                                        vm[:, 0:WG], Alu.mult)
                nc.vector.tensor_tensor(xs[0:96, 5 * WG:6 * WG],
                                        xs[0:96, 5 * WG:6 * WG],
                                        vm[:, WG:2 * WG], Alu.mult)
                # ---- halo-row duplication between blocks (DMA, partition-free)
                if stage < 9:
                    continue
                for q in range(3):
                    nc.sync.dma_start(
                        xs[32 * q:32 * q + 2, WG:],
                        xs[32 * q + 28:32 * q + 30, :(NB - 1) * WG])
                    nc.sync.dma_start(
                        xs[32 * q + 30:32 * q + 32, :(NB - 1) * WG],
                        xs[32 * q + 2:32 * q + 4, WG:])
                _ctx.__exit__(None, None, None)

            nc.sync.dma_start(d_out[:], xs[:])
    return nc


_prog_cache = {}
LAST_EXEC_S = None


def kernel(**inputs):
    cell = np.asarray(inputs["cell"], np.float32)
    food = np.asarray(inputs["food"], np.float32)
    w2 = np.asarray(inputs["w2"], np.float32)
    b2 = np.asarray(inputs["b2"], np.float32)
    w3 = np.asarray(inputs["w3"], np.float32)
    b3 = np.asarray(inputs["b3"], np.float32)
    w4 = np.asarray(inputs["w4"], np.float32)
    b4 = np.asarray(inputs["b4"], np.float32)
    steps = int(inputs["steps"])
    if steps <= 0:
        return cell.copy(), food

    from concourse.bass_utils import run_bass_kernel_spmd

    scent = _scent_host(food)
    t = _build_tables(w2, w3, w4)
    biases_zero = (not b2.any()) and (not b3.any()) and (not b4.any())
    # per-partition bias vectors in the packed layouts
    biasv = np.zeros((128, 3), np.float32)
    for co in range(8):
        biasv[16 * co:16 * co + 16, 0] = b2[co]
        biasv[16 * co:16 * co + 16, 1] = b3[co]
    for s in range(3):
        biasv[16 * s:16 * s + 16, 2] = b4[s]

    key = (steps, biases_zero)
    if key not in _prog_cache:
        nc = _build_program(steps, biases_zero, use_loop=True)
        _split_excess_waits(nc, max_waits=1)
        _prog_cache[key] = nc
    nc = _prog_cache[key]

    shared = {"L1": t["L1"], "L2": t["L2"], "L3": t["L3"], "Lres": t["Lres"],
              "Lpre": t["Lpre"], "Lpost": t["Lpost"], "Lrep": t["Lrep"],
              "Lrb": t["Lrb"], "biasv": biasv}
    in_maps = []
    for core in range(NCORES):
        m = dict(shared)
        m["xs0"] = _build_xs(cell, scent, core)
        vmc = np.zeros((96, 2, WG), np.float32)
        r0 = 128 * core - TOP
        for bi, b in enumerate((0, 5)):
            for rr in range(BR):
                g = r0 + BI * b - 2 + rr
                if 0 <= g < H:
                    for s in range(3):
                        vmc[32 * s + rr, bi, :] = 1.0
        m["vm"] = vmc.reshape(96, 2 * WG)
        in_maps.append(m)

    import time as _time
    _t0 = _time.time()
    res = run_bass_kernel_spmd(nc, in_maps, list(range(NCORES))).results
    global LAST_EXEC_S
    LAST_EXEC_S = _time.time() - _t0

    out = np.empty((4, H, W), np.float32)
    out[3] = scent
    for core in range(NCORES):
        xs = res[core]["xs_out"].reshape(128, NB, WG)
        # interior rows rr 2..29 of block b = slab rows 28b..28b+27
        slab = xs.reshape(4, 32, NB, WG)[0:3, 2:30, :, 1:1 + W]  # [3,28,NB,W]
        slab = slab.transpose(0, 2, 1, 3).reshape(3, NB * BI, W)
        out[0:3, 128 * core:128 * (core + 1)] = slab[:, TOP:TOP + 128]
    return out, food


def _split_excess_waits(nc, max_waits=1):
    import concourse.mybir as mybir
    ctr = [0]
    for bb in nc.main_func.blocks:
        i = 0
        while i < len(bb.instructions):
            ins = bb.instructions[i]
            si = ins.sync_info
            if si is not None and si.on_wait is not None and len(si.on_wait) > max_waits:
                waits = list(si.on_wait)
                keep = waits[-max_waits:]
                extra = waits[:-max_waits]
                pos = i
                for j in range(0, len(extra), max_waits):
                    chunk = extra[j:j + max_waits]
                    ctr[0] += 1
                    nop = mybir.InstNoOp(name=f"WSPLIT-{ctr[0]}", ins=[], outs=[])
                    nop.engine = ins.engine
                    nop.debug = ins.debug
                    nop.sync_info = mybir.SyncInfo(on_wait=chunk, on_update=[])
                    nc.register_instruction(nop, overwrite=True)
                    bb.instructions.insert(pos, nop)
                    pos += 1
                    i += 1
                ins.sync_info = mybir.SyncInfo(
                    on_wait=keep, on_update=list(si.on_update or []))
            i += 1


# revision 15
# speedup vs baseline: 2.1933x; 2.1933x over previous
"""Trainium2 Bass kernel for nn_Complex_CA (8-step cellular automaton).

Sharding: 8 cores x 128-row bands with 20-row ghost zones (no per-step
communication; validity shrinks 2 rows/step, margin absorbs it).

Per-core layout: 6 row-blocks of 32 (28 interior + 2+2 halo), columns padded
to 1026 with zero guards.  SBUF state tile xs[128, 6*1026]: partition =
32*ci + rr (ci: 0..2 = state channels, 3 = static scent), free = (block, col).

Per step: 3x3 conv via banded fp32r matmuls (4 ci row-tiled concurrent,
K=32 quadrant windows), 1x1 convs via (ch,row)-packed K=128 matmuls,
maxpool>0.1 via count-dilation matmuls on thresholded bits, life mask =
min(min(pre,1),post), state rebuilt to quadrant layout via permutation
matmuls, clip fused into the rebuild evacuation.
"""
import sys
import numpy as np

sys.path.insert(0, "/opt/trn_rl_repo")

H = W = 1024
NCORES = 8
NB = 6            # row blocks per core
BI = 28           # interior rows per block
BR = 32           # rows per block (with halo)
WG = 1026         # padded width (1 guard col each side)
TOP = 20          # ghost rows above the core band
FREE = NB * WG


def _build_tables(w2, w3, w4):
    """Host-built stationary matrices for all matmuls (fp32)."""
    t = {}
    # conv1: lhsT1[(h,dx,ci)] [32,128]: [rr_in, 16*co+ro] = w2[co,ci,dy,dx]
    L1 = np.zeros((2, 3, 4, 32, 128), np.float32)
    for h in range(2):
        for dx in range(3):
            for ci in range(4):
                for co in range(8):
                    for ro in range(16):
                        for dy in range(3):
                            ri = 16 * h + ro + dy - 1
                            if 0 <= ri < 32:
                                L1[h, dx, ci, ri, 16 * co + ro] = w2[co, ci, dy, dx]
    L1f = np.zeros((128, 6 * 128), np.float32)
    for h in range(2):
        for dx in range(3):
            blk = h * 3 + dx
            for ci in range(4):
                L1f[32 * ci:32 * ci + 32, 128 * blk:128 * blk + 128] = L1[h, dx, ci]
    t["L1"] = L1f

    # conv2: [128,128]: [16*ci+ri, 16*co+ro] = w3[co,ci]*d(ri==ro)
    L2 = np.zeros((128, 128), np.float32)
    for ci in range(8):
        for co in range(8):
            for r in range(16):
                L2[16 * ci + r, 16 * co + r] = w3[co, ci, 0, 0]
    t["L2"] = L2

    # conv3: [128,48]: [16*ci+ri, 16*s+ro] = w4[s,ci]*d(ri==ro), s<3
    L3 = np.zeros((128, 48), np.float32)
    for ci in range(8):
        for s in range(3):
            for r in range(16):
                L3[16 * ci + r, 16 * s + r] = w4[s, ci, 0, 0]
    t["L3"] = L3

    # residual passthrough: Lres[(h,s)] [32,48]: [rr, 16*s+ro] = d(rr==16h+ro)
    Lres = np.zeros((2, 3, 32, 48), np.float32)
    for h in range(2):
        for s in range(3):
            for ro in range(16):
                Lres[h, s, 16 * h + ro, 16 * s + ro] = 1.0
    Lresf = np.zeros((128, 96), np.float32)
    for h in range(2):
        for s in range(3):
            Lresf[32 * s:32 * s + 32, 48 * h:48 * h + 48] = Lres[h, s]
    t["Lres"] = Lresf

    # pre-mask dilate: [32,64]: [ri, 32h+ro] = 1 if |ri-(16h+ro)|<=1
    Lpre = np.zeros((32, 64), np.float32)
    for h in range(2):
        for ro in range(16):
            for dy in (-1, 0, 1):
                ri = 16 * h + ro + dy
                if 0 <= ri < 32:
                    Lpre[ri, 32 * h + ro] = 1.0
    t["Lpre"] = Lpre

    # post-mask dilate split-K: Lpost[hs] [16,64]: [ri, 32h+ro]=1 if |16hs+ri-(16h+ro)|<=1
    Lpost = np.zeros((2, 16, 64), np.float32)
    for hs in range(2):
        for h in range(2):
            for ro in range(16):
                for dy in (-1, 0, 1):
                    ri = 16 * h + ro + dy - 16 * hs
                    if 0 <= ri < 16:
                        Lpost[hs, ri, 32 * h + ro] = 1.0
    t["Lpost"] = Lpost.transpose(1, 0, 2).reshape(16, 128).copy()

    # life replicate: [16,48] = [I|I|I]
    Lrep = np.zeros((16, 48), np.float32)
    for s in range(3):
        for r in range(16):
            Lrep[r, 16 * s + r] = 1.0
    t["Lrep"] = Lrep

    # rebuild: Lrb[h] [48,128]: [16*s+ro, 32*ci+rr] = d(ci==s)*d(rr==16h+ro)
    Lrb = np.zeros((2, 48, 128), np.float32)
    for h in range(2):
        for s in range(3):
            for ro in range(16):
                Lrb[h, 16 * s + ro, 32 * s + 16 * h + ro] = 1.0
    t["Lrb"] = Lrb.transpose(1, 0, 2).reshape(48, 256).copy()
    return t


def _scent_host(food):
    f = np.asarray(food, np.float32)
    p = np.pad(f, 1)
    s = np.zeros_like(f)
    k = np.array([[0.25, 0.5, 0.25], [0.5, 1.0, 0.5], [0.25, 0.5, 0.25]], np.float32)
    for dy in range(3):
        for dx in range(3):
            s += k[dy, dx] * p[dy:dy + H, dx:dx + W]
    return s


def _build_xs(cell, scent, core):
    """Per-core initial xs [128, NB*WG] (quadrant layout, zero guards)."""
    chans = np.stack([cell[0], cell[1], cell[2], scent])  # [4,H,W]
    r0 = 128 * core - TOP
    g = r0 - 2 + BI * np.arange(NB)[:, None] + np.arange(BR)[None, :]  # [NB,BR]
    ok = (g >= 0) & (g < H)
    gc = np.clip(g, 0, H - 1)
    arr = chans[:, gc, :] * ok[None, :, :, None]          # [4,NB,BR,W]
    xs = np.zeros((4, BR, NB, WG), np.float32)
    xs[:, :, :, 1:1 + W] = arr.transpose(0, 2, 1, 3)
    return xs.reshape(128, NB * WG)


def _build_program(steps, biases_zero, stage=99, use_loop=False):
    import concourse.bass as bass
    import concourse.mybir as mybir
    import concourse.tile as tile

    dt = mybir.dt
    Alu = mybir.AluOpType
    Act = mybir.ActivationFunctionType
    f32 = dt.float32
    f32r = dt.float32r

    nc = bass.Bass("TRN2", target_bir_lowering=False)
    d_xs = nc.dram_tensor("xs0", [128, FREE], f32, kind="ExternalInput")
    d_L1 = nc.dram_tensor("L1", [128, 6 * 128], f32, kind="ExternalInput")
    d_L2 = nc.dram_tensor("L2", [128, 128], f32, kind="ExternalInput")
    d_L3 = nc.dram_tensor("L3", [128, 48], f32, kind="ExternalInput")
    d_Lres = nc.dram_tensor("Lres", [128, 96], f32, kind="ExternalInput")
    d_Lpre = nc.dram_tensor("Lpre", [32, 64], f32r, kind="ExternalInput")
    d_Lpost = nc.dram_tensor("Lpost", [16, 128], f32r, kind="ExternalInput")
    d_Lrep = nc.dram_tensor("Lrep", [16, 48], f32r, kind="ExternalInput")
    d_Lrb = nc.dram_tensor("Lrb", [48, 256], f32, kind="ExternalInput")
    d_bias = nc.dram_tensor("biasv", [128, 3], f32, kind="ExternalInput")
    d_vm = nc.dram_tensor("vm", [96, 2], f32, kind="ExternalInput")
    d_out = nc.dram_tensor("xs_out", [84, FREE], f32, kind="ExternalOutput")

    with tile.TileContext(nc) as tc:
        with tc.tile_pool(name="persist", bufs=1) as pp, \
             tc.tile_pool(name="work", bufs=2) as wp, \
             tc.tile_pool(name="psum", bufs=4, space="PSUM") as psp:
            xs = pp.tile([128, FREE], f32)
            nc.sync.dma_start(xs[:], d_xs[:])
            L1 = pp.tile([128, 6 * 128], f32)
            nc.sync.dma_start(L1[:], d_L1[:])
            L2 = pp.tile([128, 128], f32)
            nc.sync.dma_start(L2[:], d_L2[:])
            L3 = pp.tile([128, 48], f32)
            nc.sync.dma_start(L3[:], d_L3[:])
            Lres = pp.tile([128, 96], f32)
            nc.sync.dma_start(Lres[:], d_Lres[:])
            Lpre = pp.tile([32, 64], f32r)
            nc.sync.dma_start(Lpre[:], d_Lpre[:])
            Lpost = pp.tile([16, 128], f32r)
            nc.sync.dma_start(Lpost[:], d_Lpost[:])
            Lrep = pp.tile([16, 48], f32r)
            nc.sync.dma_start(Lrep[:], d_Lrep[:])
            Lrb = pp.tile([48, 256], f32)
            nc.sync.dma_start(Lrb[:], d_Lrb[:])
            bias = pp.tile([128, 3], f32)
            nc.sync.dma_start(bias[:], d_bias[:])
            vm = pp.tile([96, 2], f32)
            nc.sync.dma_start(vm[:], d_vm[:])

            b0 = pp.tile([32, FREE], f32r)       # pre-mask bits
            bp = pp.tile([16, NB * 2 * WG], f32r)  # post-mask bits (h-split rows)
            nc.vector.memset(bp[:].bitcast(f32), 0.0)

            def mm(out, lhsT, rhs, start, stop, tp=(0, 0)):
                nc.tensor.matmul(out, lhsT, rhs,
                                 start=start, stop=stop, tile_position=tp)

            import contextlib

            def step_iter():
                if use_loop:
                    return [tc.For_i(0, steps, 1)]
                return [contextlib.nullcontext() for _ in range(steps)]

            for _ctx in step_iter():
                if stage < 1:
                    break
                _ctx.__enter__()
                # ---- pre-mask bits from xs ch0 (guards stay 0 automatically)
                nc.gpsimd.tensor_scalar(b0[:], xs[0:32, :], 0.1, None, Alu.is_gt)

                for b in range(NB):
                    if stage < 2:
                        break
                    fb = b * WG
                    # ---- pre-mask count conv
                    ps_pre = psp.tile([128, 1024], f32, tag="ps")
                    for k in range(2):
                        for dx in range(3):
                            mm(ps_pre[0:64, 512 * k:512 * (k + 1)], Lpre[:],
                               b0[0:32, fb + dx + 512 * k: fb + dx + 512 * k + 512],
                               start=(dx == 0), stop=(dx == 2))
                    pre16 = wp.tile([16, 2048], f32, tag="pre16")
                    for h in range(2):
                        nc.scalar.activation(pre16[0:16, 1024 * h:1024 * (h + 1)],
                                             ps_pre[32 * h:32 * h + 16, :], Act.Copy)
                    if stage < 3:
                        continue

                    ps3s = []
                    for h in range(2):
                        # ---- conv1 (3x3, 4->8) banded, 4 ci row-tiled
                        ps1 = psp.tile([128, 1024], f32, tag="ps")
                        for k in range(2):
                            for dx in range(3):
                                col = (h * 3 + dx) * 128
                                mm(ps1[:, 512 * k:512 * (k + 1)],
                                   L1[:, col:col + 128],
                                   xs[:, fb + dx + 512 * k: fb + dx + 512 * k + 512],
                                   start=(dx == 0), stop=(dx == 2))
                        y8 = wp.tile([128, 1024], f32, tag="y8")
                        if biases_zero:
                            nc.scalar.activation(y8[:], ps1[:], Act.Relu)
                        else:
                            nc.scalar.activation(y8[:], ps1[:], Act.Relu,
                                                 bias=bias[0:128, 0:1])
                        if stage < 4:
                            ps3s.append(ps1)
                            continue
                        # ---- conv2 (1x1, 8->8)
                        ps2 = psp.tile([128, 1024], f32, tag="ps")
                        for k in range(2):
                            mm(ps2[:, 512 * k:512 * (k + 1)], L2[:],
                               y8[:, 512 * k:512 * (k + 1)], start=True, stop=True)
                        y2 = wp.tile([128, 1024], f32, tag="y2")
                        if biases_zero:
                            nc.scalar.activation(y2[:], ps2[:], Act.Relu)
                        else:
                            nc.scalar.activation(y2[:], ps2[:], Act.Relu,
                                                 bias=bias[0:128, 1:2])
                        if stage < 5:
                            ps3s.append(ps2)
                            continue
                        # ---- conv3 (1x1, 8->3) + residual passthrough
                        ps3 = psp.tile([128, 1024], f32, tag="ps")
                        for k in range(2):
                            mm(ps3[0:48, 512 * k:512 * (k + 1)], L3[:],
                               y2[:, 512 * k:512 * (k + 1)], start=True, stop=False)
                            mm(ps3[0:48, 512 * k:512 * (k + 1)],
                               Lres[:, 48 * h:48 * h + 48],
                               xs[:, fb + 1 + 512 * k: fb + 1 + 512 * k + 512],
                               start=False, stop=True)
                        # ---- post-mask bits from y3 ch0
                        nc.vector.tensor_scalar(
                            bp[0:16, (b * 2 + h) * WG + 1:(b * 2 + h) * WG + 1025],
                            ps3[0:16, :], 0.1, None, Alu.is_gt)
                        ps3s.append(ps3)
                    if stage < 6:
                        continue

                    # ---- post-mask count conv (split-K over h-halves)
                    ps_post = psp.tile([128, 1024], f32, tag="ps")
                    fb2 = b * 2 * WG
                    for k in range(2):
                        for dx in range(3):
                            for hs in range(2):
                                mm(ps_post[0:64, 512 * k:512 * (k + 1)],
                                   Lpost[:, 64 * hs:64 * hs + 64],
                                   bp[0:16, fb2 + hs * WG + dx + 512 * k:
                                      fb2 + hs * WG + dx + 512 * k + 512],
                                   start=(dx == 0 and hs == 0),
                                   stop=(dx == 2 and hs == 1))
                    # ---- life = min(min(pre,1), post)  in {0,1}
                    life16 = wp.tile([16, 2048], f32r, tag="life16")
                    for h in range(2):
                        nc.vector.scalar_tensor_tensor(
                            life16[0:16, 1024 * h:1024 * (h + 1)],
                            pre16[0:16, 1024 * h:1024 * (h + 1)], 1.0,
                            ps_post[32 * h:32 * h + 16, :], Alu.min, Alu.min)
                    if stage < 7:
                        continue
                    # ---- replicate life across 3 channels via PE
                    xn = wp.tile([48, 2048], f32, tag="xn")
                    for h in range(2):
                        ps_l = psp.tile([128, 1024], f32, tag="ps")
                        for k in range(2):
                            mm(ps_l[0:48, 512 * k:512 * (k + 1)], Lrep[:],
                               life16[0:16, 1024 * h + 512 * k:
                                      1024 * h + 512 * k + 512],
                               start=True, stop=True)
                        lf = wp.tile([48, 1024], f32, tag="lf")
                        nc.scalar.activation(lf[:], ps_l[0:48, :], Act.Copy)
                        # ---- apply: xn = (ps3 + b4) * life
                        if biases_zero:
                            nc.vector.scalar_tensor_tensor(
                                xn[0:48, 1024 * h:1024 * (h + 1)],
                                ps3s[h][0:48, :], 1.0, lf[:], Alu.mult, Alu.mult)
                        else:
                            nc.vector.scalar_tensor_tensor(
                                xn[0:48, 1024 * h:1024 * (h + 1)],
                                ps3s[h][0:48, :], bias[0:48, 2:3], lf[:],
                                Alu.add, Alu.mult)
                    if stage < 8:
                        continue
                    # ---- rebuild xs quadrant layout via permutation matmuls
                    ps_x = psp.tile([128, 1024], f32, tag="ps")
                    for k in range(2):
                        for h in range(2):
                            mm(ps_x[:, 512 * k:512 * (k + 1)],
                               Lrb[:, 128 * h:128 * (h + 1)],
                               xn[0:48, 1024 * h + 512 * k:1024 * h + 512 * k + 512],
                               start=(h == 0), stop=(h == 1))
                    # clip fused into the evacuation
                    nc.vector.tensor_scalar(xs[0:96, fb + 1:fb + 1025],
                                            ps_x[0:96, :], 10.0, -10.0,
                                            Alu.min, Alu.max)

                # ---- zero ghost rows at the global image boundary (edge blocks)
                nc.vector.tensor_scalar(xs[0:96, 0:WG], xs[0:96, 0:WG],
                                        vm[:, 0:1], None, Alu.mult)
                nc.vector.tensor_scalar(xs[0:96, 5 * WG:6 * WG],
                                        xs[0:96, 5 * WG:6 * WG],
                                        vm[:, 1:2], None, Alu.mult)
                # ---- halo-row duplication between blocks (DMA, partition-free)
                if stage < 9:
                    continue
                for q in range(3):
                    nc.sync.dma_start(
                        xs[32 * q:32 * q + 2, WG:],
                        xs[32 * q + 28:32 * q + 30, :(NB - 1) * WG])
                    nc.sync.dma_start(
                        xs[32 * q + 30:32 * q + 32, :(NB - 1) * WG],
                        xs[32 * q + 2:32 * q + 4, WG:])
                _ctx.__exit__(None, None, None)

            for q in range(3):
                nc.sync.dma_start(d_out[28 * q:28 * q + 28, :],
                                  xs[32 * q + 2:32 * q + 30, :])
    return nc


_prog_cache = {}
LAST_EXEC_S = None


def kernel(**inputs):
    cell = np.asarray(inputs["cell"], np.float32)
    food = np.asarray(inputs["food"], np.float32)
    w2 = np.asarray(inputs["w2"], np.float32)
    b2 = np.asarray(inputs["b2"], np.float32)
    w3 = np.asarray(inputs["w3"], np.float32)
    b3 = np.asarray(inputs["b3"], np.float32)
    w4 = np.asarray(inputs["w4"], np.float32)
    b4 = np.asarray(inputs["b4"], np.float32)
    steps = int(inputs["steps"])
    if steps <= 0:
        return cell.copy(), food

    from concourse.bass_utils import run_bass_kernel_spmd

    scent = _scent_host(food)
    t = _build_tables(w2, w3, w4)
    biases_zero = (not b2.any()) and (not b3.any()) and (not b4.any())
    # per-partition bias vectors in the packed layouts
    biasv = np.zeros((128, 3), np.float32)
    for co in range(8):
        biasv[16 * co:16 * co + 16, 0] = b2[co]
        biasv[16 * co:16 * co + 16, 1] = b3[co]
    for s in range(3):
        biasv[16 * s:16 * s + 16, 2] = b4[s]

    key = (steps, biases_zero)
    if key not in _prog_cache:
        nc = _build_program(steps, biases_zero, use_loop=True)
        _split_excess_waits(nc, max_waits=1)
        _prog_cache[key] = nc
    nc = _prog_cache[key]

    shared = {"L1": t["L1"], "L2": t["L2"], "L3": t["L3"], "Lres": t["Lres"],
              "Lpre": t["Lpre"], "Lpost": t["Lpost"], "Lrep": t["Lrep"],
              "Lrb": t["Lrb"], "biasv": biasv}
    in_maps = []
    for core in range(NCORES):
        m = dict(shared)
        m["xs0"] = _build_xs(cell, scent, core)
        vmc = np.zeros((96, 2), np.float32)
        r0 = 128 * core - TOP
        for bi, b in enumerate((0, 5)):
            for rr in range(BR):
                g = r0 + BI * b - 2 + rr
                if 0 <= g < H:
                    for s in range(3):
                        vmc[32 * s + rr, bi] = 1.0
        m["vm"] = vmc
        in_maps.append(m)

    import time as _time
    _t0 = _time.time()
    res = run_bass_kernel_spmd(nc, in_maps, list(range(NCORES))).results
    global LAST_EXEC_S
    LAST_EXEC_S = _time.time() - _t0

    out = np.empty((4, H, W), np.float32)
    out[3] = scent
    for core in range(NCORES):
        xs = res[core]["xs_out"].reshape(3, BI, NB, WG)
        slab = xs[:, :, :, 1:1 + W]                               # [3,28,NB,W]
        slab = slab.transpose(0, 2, 1, 3).reshape(3, NB * BI, W)
        out[0:3, 128 * core:128 * (core + 1)] = slab[:, TOP:TOP + 128]
    return out, food


def _split_excess_waits(nc, max_waits=1):
    import concourse.mybir as mybir
    ctr = [0]
    for bb in nc.main_func.blocks:
        i = 0
        while i < len(bb.instructions):
            ins = bb.instructions[i]
            si = ins.sync_info
            if si is not None and si.on_wait is not None and len(si.on_wait) > max_waits:
                waits = list(si.on_wait)
                keep = waits[-max_waits:]
                extra = waits[:-max_waits]
                pos = i
                for j in range(0, len(extra), max_waits):
                    chunk = extra[j:j + max_waits]
                    ctr[0] += 1
                    nop = mybir.InstNoOp(name=f"WSPLIT-{ctr[0]}", ins=[], outs=[])
                    nop.engine = ins.engine
                    nop.debug = ins.debug
                    nop.sync_info = mybir.SyncInfo(on_wait=chunk, on_update=[])
                    nc.register_instruction(nop, overwrite=True)
                    bb.instructions.insert(pos, nop)
                    pos += 1
                    i += 1
                ins.sync_info = mybir.SyncInfo(
                    on_wait=keep, on_update=list(si.on_update or []))
            i += 1
